# revision 24
# baseline (speedup 1.0000x reference)
"""Trainium2 Bass kernel for nn_InterfaceGraph (retrieval_knn).

Segment-restricted nearest-neighbor DISTANCES between pos_a and pos_b
(16384 x 16384 pairwise, block-diagonal over 64 sorted graphs), sharded
over 8 NeuronCores (8 graphs per core).

Key observation: the reference returns only (mask, dists) -- no
indices.  dist = sqrt(min d^2), and min d^2 = -max(2 a.b - |b|^2 -
|a|^2), so the row-max VALUE of the K=24 bf16-split matmul is the
answer; no argmax index extraction is needed at all.

Per 128-row tile, the matmul writes -d^2 into PSUM (f32).  The idle
scalar engine stages PSUM -> SBUF fp16 in large grouped copies; the
vector engine then reduces with an fp16 fold tree (tensor_tensor max
runs in the DVE 2x packed mode) plus one 1x tensor_reduce on the
37-wide tail -- ~0.59 ns/col instead of 2.08 ns/col for the
max8+find_index8 scheme.  A minority of tile-groups skip staging and
reduce directly from PSUM with MAX8 on the DVE, balancing the scalar
and vector pipelines.

fp16 rounding gives ~5e-4 relative dist error (tolerance 2e-2).
Atoms whose estimated dist lies within 0.05 of the 10.0 interface
cutoff are recomputed exactly on host (a few hundred rows), which
also makes the cutoff mask bit-exact.
"""

import numpy as np
import ml_dtypes

NCORES = 8
G = 64
GPC = G // NCORES
NUM_RESIDUES = 2048
CUTOFF = np.float32(10.0)
BIG = np.float32(2.0 ** 26)
K = 24            # 18 cross rows + 3 |b|^2 rows + 3 |a|^2 rows
GSZ = 4           # tiles per psum group, one 2KB psum bank per tile
PSB = 512         # f32 per psum bank (matmul output must not cross banks)
PATCH_BAND = 0.05
DIRECT_FRAC = 0.0    # fraction of tile-groups reduced straight from PSUM
NWARM = 0         # dummy matmuls before the real stream (HAM never opens
                  # on this part -- K stays 4/8 -- so warmup only delays)

PROFILE = False
LAST_EXEC_NS = None

BF16 = ml_dtypes.bfloat16

_prog_cache = {}


def _round_up(x, m):
    return (x + m - 1) // m * m


def _install_ntff_hook():
    import sys
    import types
    if 'antenv.axon_hooks' in sys.modules:
        return
    from trn_agent_boot.trn_boot import _ntff_profile_via_ctypes
    hook = _ntff_profile_via_ctypes('/opt/axon/libaxon_pjrt.so')
    mod = types.ModuleType('antenv.axon_hooks')
    mod.get_axon_ntff_profile_hook = lambda: hook
    sys.modules['antenv.axon_hooks'] = mod


def _split3(v):
    """bf16x3 split: v ~= v1 + v2 + v3 with ~24-bit mantissa coverage."""
    v = v.astype(np.float32)
    v1 = v.astype(BF16).astype(np.float32)
    r = v - v1
    v2 = r.astype(BF16).astype(np.float32)
    v3 = (r - v2).astype(BF16).astype(np.float32)
    return v1, v2, v3


class _Geom:
    """Per-slot shapes shared by all cores (SPMD program is one program).

    Row tiling is slot-sorted per side (graphs sorted by row count so
    per-slot tile counts stay tight across cores).  The reduction
    window is a single uniform width WSTAR for every slot, which lets
    PSUM tiles group 6 matmuls and the fold tree run as a handful of
    wide strided ops per group.
    """

    def __init__(self, na, nb):
        assign = self._assign(na, nb)          # [core, i] -> graph id
        self.graphA = np.zeros((NCORES, GPC), dtype=np.int64)
        self.graphB = np.zeros((NCORES, GPC), dtype=np.int64)
        for c in range(NCORES):
            gs = assign[c]
            self.graphA[c] = gs[np.argsort(-na[gs], kind="stable")]
            self.graphB[c] = gs[np.argsort(-nb[gs], kind="stable")]
        na_A = na[self.graphA]
        nb_B = nb[self.graphB]
        self.TA = [int(-(-na_A[:, s].max() // 128)) for s in range(GPC)]
        self.TB = [int(-(-nb_B[:, s].max() // 128)) for s in range(GPC)]
        self.baseTA = np.concatenate([[0], np.cumsum(self.TA)]).astype(int)
        self.baseTB = np.concatenate([[0], np.cumsum(self.TB)]).astype(int)
        # uniform fold window: multiple of 8 so 3 halvings stay integral
        self.WSTAR = int(_round_up(int(max(na.max(), nb.max())), 8))
        # slot id for each tile index
        self.slotA = sum(([s] * self.TA[s] for s in range(GPC)), [])
        self.slotB = sum(([s] * self.TB[s] for s in range(GPC)), [])

    @staticmethod
    def _assign(na, nb):
        """Deterministic graph->core assignment minimizing total row-tile
        count (the per-slot cross-core maxima on both sides)."""
        def cost(a):
            A = np.sort(na[a], axis=1)[:, ::-1]
            B = np.sort(nb[a], axis=1)[:, ::-1]
            ta = sum(int(-(-A[:, s].max() // 128)) for s in range(GPC))
            tb = sum(int(-(-B[:, s].max() // 128)) for s in range(GPC))
            return ta + tb
        order = np.argsort(-na, kind="stable")
        best = np.zeros((NCORES, GPC), dtype=np.int64)
        for r, g in enumerate(order):        # na-balanced snake deal
            c = r % NCORES if (r // NCORES) % 2 == 0 else \
                NCORES - 1 - r % NCORES
            best[c, r // NCORES] = g
        bcost = cost(best)
        rng = np.random.default_rng(0)
        for _ in range(8000):                # swap search (deterministic)
            c1, c2 = rng.integers(0, NCORES, 2)
            i1, i2 = rng.integers(0, GPC, 2)
            a = best.copy()
            a[c1, i1], a[c2, i2] = a[c2, i2], a[c1, i1]
            ac = cost(a)
            if ac <= bcost:
                best, bcost = a, ac
        return best

    def groups(self, side):
        """[(kk0, gn, direct)] covering all tiles of a side."""
        tot = int(self.baseTA[-1] if side == "A" else self.baseTB[-1])
        out = []
        kk0 = 0
        while kk0 < tot:
            gn = min(GSZ, tot - kk0)
            out.append([kk0, gn, False])
            kk0 += gn
        # The last group of side B is reduced straight from PSUM with
        # MAX8: its reduction then overlaps the matmul stream instead of
        # trailing it with a copy + fold chain (the DVE is idle by then).
        if side == "B":
            out[-1][2] = True
        ndirect = max(0, int(round(DIRECT_FRAC * len(out))))
        for i in range(len(out) - ndirect, len(out)):
            out[i][2] = True
        return [tuple(g) for g in out]

    def key(self):
        return (tuple(self.TA), tuple(self.TB), self.WSTAR)


def _build_program(geom):
    from contextlib import ExitStack

    import concourse.bacc as bacc
    import concourse.mybir as mybir
    import concourse.tile as tile

    f32 = mybir.dt.float32
    f16 = mybir.dt.float16
    bf16 = mybir.dt.bfloat16
    Alu = mybir.AluOpType
    Ax = mybir.AxisListType

    W = geom.WSTAR
    H1, H2, H3 = W // 2, W // 4, W // 8
    LA = int(geom.baseTA[-1]) * 128
    LB = int(geom.baseTB[-1]) * 128
    TAtot = int(geom.baseTA[-1])
    TBtot = int(geom.baseTB[-1])
    RW = GPC * W

    nc = bacc.Bacc("TRN2", target_bir_lowering=False, debug=False,
                   enable_asserts=True, num_devices=NCORES)

    lhsA = nc.dram_tensor("lhsA", [K, LA], bf16, kind="ExternalInput").ap()
    rhsB = nc.dram_tensor("rhsB", [K, RW], bf16, kind="ExternalInput").ap()
    lhsB = nc.dram_tensor("lhsB", [K, LB], bf16, kind="ExternalInput").ap()
    rhsA = nc.dram_tensor("rhsA", [K, RW], bf16, kind="ExternalInput").ap()
    dirA = any(d for _, _, d in geom.groups("A"))
    dirB = any(d for _, _, d in geom.groups("B"))
    valA = nc.dram_tensor("valA", [128, TAtot], f32, kind="ExternalOutput").ap()
    valB = nc.dram_tensor("valB", [128, TBtot], f32, kind="ExternalOutput").ap()
    if dirA:
        v8A = nc.dram_tensor("v8A", [128, TAtot * 8], f32,
                             kind="ExternalOutput").ap()
    if dirB:
        v8B = nc.dram_tensor("v8B", [128, TBtot * 8], f32,
                             kind="ExternalOutput").ap()

    with tile.TileContext(nc) as tc:
        with ExitStack() as ctx:
            const = ctx.enter_context(tc.tile_pool(name="const", bufs=1))
            psum = ctx.enter_context(
                tc.tile_pool(name="psum", bufs=2, space="PSUM"))
            work = ctx.enter_context(tc.tile_pool(name="work", bufs=2))

            # Input DMAs split across both hardware queues, head chunks
            # first so the opening matmul groups can start ~3us earlier
            # than a whole-tensor transfer would allow.
            HCH = min(GSZ * 2 * 128, LA)       # lhsA head: first 2 groups
            HRW = min(4 * W, RW)               # rhsB head: first 4 slots
            lhsA_sb = const.tile([K, LA], bf16, tag="lhsA")
            nc.sync.dma_start(lhsA_sb[:, 0:HCH], lhsA[:, 0:HCH])
            rhsB_sb = const.tile([K, RW], bf16, tag="rhsB")
            nc.scalar.dma_start(rhsB_sb[:, 0:HRW], rhsB[:, 0:HRW])
            nc.sync.dma_start(lhsA_sb[:, HCH:], lhsA[:, HCH:])
            nc.scalar.dma_start(rhsB_sb[:, HRW:], rhsB[:, HRW:])
            lhsB_sb = const.tile([K, LB], bf16, tag="lhsB")
            nc.sync.dma_start(lhsB_sb[:], lhsB[:])
            rhsA_sb = const.tile([K, RW], bf16, tag="rhsA")
            nc.scalar.dma_start(rhsA_sb[:], rhsA[:])

            if NWARM:
                warm_sb = const.tile([K, PSB], bf16, tag="warm")
                nc.vector.memset(warm_sb[:], 0)
                wps = psum.tile([128, GSZ, PSB], f32, tag="ps")
                for i in range(NWARM):
                    nc.tensor.matmul(
                        wps[:, i % GSZ, :], warm_sb[:, 0:128], warm_sb[:],
                        start=True, stop=True)

            SA = const.tile([128, TAtot, W], f16, tag="SA")
            SB = const.tile([128, TBtot, W], f16, tag="SB")
            valA_sb = const.tile([128, TAtot], f32, tag="valA")
            valB_sb = const.tile([128, TBtot], f32, tag="valB")
            v8A_sb = None
            v8B_sb = None
            if dirA:
                v8A_sb = const.tile([128, TAtot * 8], f32, tag="v8A")
            if dirB:
                v8B_sb = const.tile([128, TBtot * 8], f32, tag="v8B")

            def side(side_name, lhs_sb, rhs_sb, slot_of, S, val_sb, v8_sb):
                for kk0, gn, direct in geom.groups(side_name):
                    ps = psum.tile([128, GSZ, PSB], f32, tag="ps")
                    for i in range(gn):
                        kk = kk0 + i
                        s = slot_of[kk]
                        nc.tensor.matmul(
                            ps[:, i, 0:W],
                            lhs_sb[:, kk * 128:(kk + 1) * 128],
                            rhs_sb[:, s * W:(s + 1) * W],
                            start=True, stop=True)
                    if direct:
                        for i in range(gn):
                            kk = kk0 + i
                            nc.vector.max(v8_sb[:, kk * 8:(kk + 1) * 8],
                                          ps[:, i, 0:W])
                    else:
                        nc.scalar.activation(
                            S[:, kk0:kk0 + gn, :], ps[:, 0:gn, 0:W],
                            mybir.ActivationFunctionType.Copy)
                        b1 = work.tile([128, GSZ, H1], f16, tag="b1")
                        nc.vector.tensor_tensor(
                            b1[:, 0:gn, :],
                            S[:, kk0:kk0 + gn, 0:H1],
                            S[:, kk0:kk0 + gn, H1:W], op=Alu.max)
                        b2 = work.tile([128, GSZ, H2], f16, tag="b2")
                        nc.vector.tensor_tensor(
                            b2[:, 0:gn, :],
                            b1[:, 0:gn, 0:H2],
                            b1[:, 0:gn, H2:H1], op=Alu.max)
                        nc.vector.tensor_reduce(
                            val_sb[:, kk0:kk0 + gn], b2[:, 0:gn, :],
                            Ax.X, Alu.max)

            side("A", lhsA_sb, rhsB_sb, geom.slotA, SA, valA_sb, v8A_sb)
            side("B", lhsB_sb, rhsA_sb, geom.slotB, SB, valB_sb, v8B_sb)

            nc.sync.dma_start(valA[:], valA_sb[:])
            hb = max(1, TBtot - GSZ)
            nc.sync.dma_start(valB[:, 0:hb], valB_sb[:, 0:hb])
            nc.sync.dma_start(valB[:, hb:], valB_sb[:, hb:])
            if dirA:
                nc.sync.dma_start(v8A[:], v8A_sb[:])
            if dirB:
                nc.sync.dma_start(v8B[:], v8B_sb[:])

    nc.compile()
    return nc


def _pack_side(pos_row, pos_col, starts_row, starts_col, graphs, baseT, W):
    """lhs/rhs bf16 packs for one core, one direction.

    PSUM value = 2 p.q - |q|^2 - |p|^2 = -d^2.
    K-row order: tier-2 (smallest) first, tier-0 last, |p|^2 rows last.
    """
    LT = int(baseT[-1]) * 128
    lhs = np.zeros((K, LT), dtype=np.float32)
    rhs = np.zeros((K, GPC * W), dtype=np.float32)
    #  rows 0-8   : tier2 cross (c,x3) lhs a1,a2,a3 / rhs b3,b2,b1
    #  row  9     : tier2 -q3      (lhs -1, rhs q3)
    #  rows 10-15 : tier1 cross    lhs a1,a2 / rhs b2,b1
    #  row  16    : tier1 -q2
    #  rows 17-19 : tier0 cross    lhs a1 / rhs b1
    #  row  20    : tier0 -q1  (+BIG on padding -> pad col = -BIG)
    #  rows 21-23 : -|p|^2 tiers (lhs p-squared splits, rhs -1)
    lhs[9, :] = -1.0
    lhs[16, :] = -1.0
    lhs[20, :] = -1.0
    rhs[20, :] = BIG
    tb = 0
    for s in range(GPC):
        g = graphs[s]
        p = pos_row[starts_row[g]:starts_row[g + 1]]
        n = p.shape[0]
        lb = int(baseT[s]) * 128
        for c in range(3):
            a1, a2, a3 = _split3(np.float32(2.0) * p[:, c])
            lhs[0 + c * 3, lb:lb + n] = a1
            lhs[1 + c * 3, lb:lb + n] = a2
            lhs[2 + c * 3, lb:lb + n] = a3
            lhs[10 + c * 2, lb:lb + n] = a1
            lhs[11 + c * 2, lb:lb + n] = a2
            lhs[17 + c, lb:lb + n] = a1
        pp = (p[:, 0] * p[:, 0] + p[:, 1] * p[:, 1]) + p[:, 2] * p[:, 2]
        p1, p2, p3 = _split3(pp)
        lhs[21, lb:lb + n] = p3
        lhs[22, lb:lb + n] = p2
        lhs[23, lb:lb + n] = p1

        q = pos_col[starts_col[g]:starts_col[g + 1]]
        m = q.shape[0]
        rb = s * W
        qq = (q[:, 0] * q[:, 0] + q[:, 1] * q[:, 1]) + q[:, 2] * q[:, 2]
        q1, q2, q3 = _split3(qq)
        for c in range(3):
            b1, b2, b3 = _split3(q[:, c])
            rhs[0 + c * 3, rb:rb + m] = b3
            rhs[1 + c * 3, rb:rb + m] = b2
            rhs[2 + c * 3, rb:rb + m] = b1
            rhs[10 + c * 2, rb:rb + m] = b2
            rhs[11 + c * 2, rb:rb + m] = b1
            rhs[17 + c, rb:rb + m] = b1
        rhs[9, rb:rb + m] = q3
        rhs[16, rb:rb + m] = q2
        rhs[20, rb:rb + m] = q1
        rhs[21, rb:rb + m] = -1.0
        rhs[22, rb:rb + m] = -1.0
        rhs[23, rb:rb + m] = -1.0
        tb += 1
    return lhs.astype(BF16), rhs.astype(BF16)


def _unpack_side(val, v8, groups, starts_row, graphs, TA, baseT, dist_full):
    direct_kks = set()
    for kk0, gn, direct in groups:
        if direct:
            direct_kks.update(range(kk0, kk0 + gn))
    for s in range(GPC):
        g = graphs[s]
        n = starts_row[g + 1] - starts_row[g]
        for t in range((n + 127) // 128):
            rows = min(128, n - t * 128)
            kk = int(baseT[s]) + t
            if kk in direct_kks:
                v = v8[:rows, kk * 8].astype(np.float64)
            else:
                v = val[:rows, kk].astype(np.float64)
            atoms = starts_row[g] + t * 128 + np.arange(rows)
            dist_full[atoms] = np.sqrt(np.maximum(-v, 0.0))


def _patch_rows(pos_row, pos_col, starts_row, starts_col, n2g_row,
                dist_full):
    """Exact recompute (reference arithmetic) for rows near the cutoff."""
    sel = np.nonzero(np.abs(dist_full - 10.0) < PATCH_BAND)[0]
    for r in sel:
        g = int(n2g_row[r])
        Q = pos_col[starts_col[g]:starts_col[g + 1]]
        p = pos_row[r]
        d2 = ((p[0] - Q[:, 0]) ** 2 + (p[1] - Q[:, 1]) ** 2
              + (p[2] - Q[:, 2]) ** 2)
        j = int(np.argmin(d2))
        d = p - Q[j]
        dist_full[r] = np.sqrt((d[0] * d[0] + d[1] * d[1]) + d[2] * d[2])


def kernel(pos_a, pos_b, node2graph_a, node2graph_b,
           atom2residue_a, atom2residue_b, is_mutation):
    global LAST_EXEC_NS

    from concourse.bass_utils import run_bass_kernel_spmd

    pos_a = np.asarray(pos_a, dtype=np.float32)
    pos_b = np.asarray(pos_b, dtype=np.float32)
    node2graph_a = np.asarray(node2graph_a)
    node2graph_b = np.asarray(node2graph_b)
    atom2residue_a = np.asarray(atom2residue_a)
    atom2residue_b = np.asarray(atom2residue_b)
    is_mutation = np.asarray(is_mutation)

    Na = pos_a.shape[0]
    Nb = pos_b.shape[0]

    sa = np.searchsorted(node2graph_a, np.arange(G + 1)).astype(np.int64)
    sb = np.searchsorted(node2graph_b, np.arange(G + 1)).astype(np.int64)
    na = np.diff(sa)
    nb = np.diff(sb)
    assert na.min() > 0 and nb.min() > 0, "empty graph block not supported"

    geom = _Geom(na, nb)
    key = geom.key()
    if key not in _prog_cache:
        _prog_cache[key] = _build_program(geom)
    nc = _prog_cache[key]

    in_maps = []
    for c in range(NCORES):
        lhsA, rhsB = _pack_side(pos_a, pos_b, sa, sb, geom.graphA[c],
                                geom.baseTA, geom.WSTAR)
        lhsB, rhsA = _pack_side(pos_b, pos_a, sb, sa, geom.graphB[c],
                                geom.baseTB, geom.WSTAR)
        in_maps.append({"lhsA": lhsA, "rhsB": rhsB,
                        "lhsB": lhsB, "rhsA": rhsA})

    if PROFILE:
        _install_ntff_hook()
    res = run_bass_kernel_spmd(nc, in_maps, list(range(NCORES)),
                               trace=bool(PROFILE))
    if PROFILE:
        LAST_EXEC_NS = res.exec_time_ns

    dist_a = np.zeros(Na, dtype=np.float64)
    dist_b = np.zeros(Nb, dtype=np.float64)
    gA = geom.groups("A")
    gB = geom.groups("B")
    for c in range(NCORES):
        _unpack_side(res.results[c]["valA"], res.results[c].get("v8A"), gA,
                     sa, geom.graphA[c], geom.TA, geom.baseTA, dist_a)
        _unpack_side(res.results[c]["valB"], res.results[c].get("v8B"), gB,
                     sb, geom.graphB[c], geom.TB, geom.baseTB, dist_b)

    dist_a = dist_a.astype(np.float32)
    dist_b = dist_b.astype(np.float32)
    _patch_rows(pos_a, pos_b, sa, sb, node2graph_a, dist_a)
    _patch_rows(pos_b, pos_a, sb, sa, node2graph_b, dist_b)

    def iface_mask(dist, atom2residue):
        is_if = (dist < CUTOFF).astype(np.int32)
        res_max = np.zeros(NUM_RESIDUES, dtype=np.int32)
        np.maximum.at(res_max, atom2residue, is_if)
        return res_max[atom2residue] > 0

    mask_a = iface_mask(dist_a, atom2residue_a)
    mask_b = iface_mask(dist_b, atom2residue_b)
    mask = np.concatenate([mask_a, mask_b]) | is_mutation.astype(bool)
    dists = np.concatenate([dist_a, dist_b]).astype(np.float32)
    return mask, dists


# revision 25
# speedup vs baseline: 1.0318x; 1.0318x over previous
"""Trainium2 Bass kernel for nn_InterfaceGraph (retrieval_knn).

Segment-restricted nearest-neighbor DISTANCES between pos_a and pos_b
(16384 x 16384 pairwise, block-diagonal over 64 sorted graphs), sharded
over 8 NeuronCores (8 graphs per core).

Key observation: the reference returns only (mask, dists) -- no
indices.  dist = sqrt(min d^2), and min d^2 = -max(2 a.b - |b|^2 -
|a|^2), so the row-max VALUE of the K=24 bf16-split matmul is the
answer; no argmax index extraction is needed at all.

Per 128-row tile, the matmul writes -d^2 into PSUM (f32).  The idle
scalar engine stages PSUM -> SBUF fp16 in large grouped copies; the
vector engine then reduces with an fp16 fold tree (tensor_tensor max
runs in the DVE 2x packed mode) plus one 1x tensor_reduce on the
37-wide tail -- ~0.59 ns/col instead of 2.08 ns/col for the
max8+find_index8 scheme.  A minority of tile-groups skip staging and
reduce directly from PSUM with MAX8 on the DVE, balancing the scalar
and vector pipelines.

fp16 rounding gives ~5e-4 relative dist error (tolerance 2e-2).
Atoms whose estimated dist lies within 0.05 of the 10.0 interface
cutoff are recomputed exactly on host (a few hundred rows), which
also makes the cutoff mask bit-exact.
"""

import numpy as np
import ml_dtypes

NCORES = 8
G = 64
GPC = G // NCORES
NUM_RESIDUES = 2048
CUTOFF = np.float32(10.0)
BIG = np.float32(2.0 ** 26)
K = 24            # 18 cross rows + 3 |b|^2 rows + 3 |a|^2 rows
GSZ = 4           # tiles per psum group, one 2KB psum bank per tile
PSB = 512         # f32 per psum bank (matmul output must not cross banks)
PATCH_BAND = 0.05
DIRECT_FRAC = 0.0    # fraction of tile-groups reduced straight from PSUM
NWARM = 0         # dummy matmuls before the real stream (HAM never opens
                  # on this part -- K stays 4/8 -- so warmup only delays)

PROFILE = False
LAST_EXEC_NS = None

BF16 = ml_dtypes.bfloat16

_prog_cache = {}


def _round_up(x, m):
    return (x + m - 1) // m * m


def _install_ntff_hook():
    import sys
    import types
    if 'antenv.axon_hooks' in sys.modules:
        return
    from trn_agent_boot.trn_boot import _ntff_profile_via_ctypes
    hook = _ntff_profile_via_ctypes('/opt/axon/libaxon_pjrt.so')
    mod = types.ModuleType('antenv.axon_hooks')
    mod.get_axon_ntff_profile_hook = lambda: hook
    sys.modules['antenv.axon_hooks'] = mod


def _split3(v):
    """bf16x3 split: v ~= v1 + v2 + v3 with ~24-bit mantissa coverage."""
    v = v.astype(np.float32)
    v1 = v.astype(BF16).astype(np.float32)
    r = v - v1
    v2 = r.astype(BF16).astype(np.float32)
    v3 = (r - v2).astype(BF16).astype(np.float32)
    return v1, v2, v3


class _Geom:
    """Per-slot shapes shared by all cores (SPMD program is one program).

    Row tiling is slot-sorted per side (graphs sorted by row count so
    per-slot tile counts stay tight across cores).  The reduction
    window is a single uniform width WSTAR for every slot, which lets
    PSUM tiles group 6 matmuls and the fold tree run as a handful of
    wide strided ops per group.
    """

    def __init__(self, na, nb):
        assign = self._assign(na, nb)          # [core, i] -> graph id
        self.graphA = np.zeros((NCORES, GPC), dtype=np.int64)
        self.graphB = np.zeros((NCORES, GPC), dtype=np.int64)
        for c in range(NCORES):
            gs = assign[c]
            self.graphA[c] = gs[np.argsort(-na[gs], kind="stable")]
            self.graphB[c] = gs[np.argsort(-nb[gs], kind="stable")]
        na_A = na[self.graphA]
        nb_B = nb[self.graphB]
        self.TA = [int(-(-na_A[:, s].max() // 128)) for s in range(GPC)]
        self.TB = [int(-(-nb_B[:, s].max() // 128)) for s in range(GPC)]
        self.baseTA = np.concatenate([[0], np.cumsum(self.TA)]).astype(int)
        self.baseTB = np.concatenate([[0], np.cumsum(self.TB)]).astype(int)
        # uniform fold window: multiple of 8 so 3 halvings stay integral
        self.WSTAR = int(_round_up(int(max(na.max(), nb.max())), 8))
        # slot id for each tile index
        self.slotA = sum(([s] * self.TA[s] for s in range(GPC)), [])
        self.slotB = sum(([s] * self.TB[s] for s in range(GPC)), [])

    @staticmethod
    def _assign(na, nb):
        """Deterministic graph->core assignment minimizing total row-tile
        count (the per-slot cross-core maxima on both sides)."""
        def cost(a):
            A = np.sort(na[a], axis=1)[:, ::-1]
            B = np.sort(nb[a], axis=1)[:, ::-1]
            ta = sum(int(-(-A[:, s].max() // 128)) for s in range(GPC))
            tb = sum(int(-(-B[:, s].max() // 128)) for s in range(GPC))
            return ta + tb
        order = np.argsort(-na, kind="stable")
        best = np.zeros((NCORES, GPC), dtype=np.int64)
        for r, g in enumerate(order):        # na-balanced snake deal
            c = r % NCORES if (r // NCORES) % 2 == 0 else \
                NCORES - 1 - r % NCORES
            best[c, r // NCORES] = g
        bcost = cost(best)
        rng = np.random.default_rng(0)
        for _ in range(8000):                # swap search (deterministic)
            c1, c2 = rng.integers(0, NCORES, 2)
            i1, i2 = rng.integers(0, GPC, 2)
            a = best.copy()
            a[c1, i1], a[c2, i2] = a[c2, i2], a[c1, i1]
            ac = cost(a)
            if ac <= bcost:
                best, bcost = a, ac
        return best

    def groups(self, side):
        """[(kk0, gn, direct)] covering all tiles of a side."""
        tot = int(self.baseTA[-1] if side == "A" else self.baseTB[-1])
        out = []
        kk0 = 0
        while kk0 < tot:
            gn = min(GSZ, tot - kk0)
            out.append([kk0, gn, False])
            kk0 += gn
        ndirect = max(0, int(round(DIRECT_FRAC * len(out))))
        for i in range(len(out) - ndirect, len(out)):
            out[i][2] = True
        return [tuple(g) for g in out]

    def key(self):
        return (tuple(self.TA), tuple(self.TB), self.WSTAR)


def _build_program(geom):
    from contextlib import ExitStack

    import concourse.bacc as bacc
    import concourse.mybir as mybir
    import concourse.tile as tile

    f32 = mybir.dt.float32
    f16 = mybir.dt.float16
    bf16 = mybir.dt.bfloat16
    Alu = mybir.AluOpType
    Ax = mybir.AxisListType

    W = geom.WSTAR
    H1, H2, H3 = W // 2, W // 4, W // 8
    LA = int(geom.baseTA[-1]) * 128
    LB = int(geom.baseTB[-1]) * 128
    TAtot = int(geom.baseTA[-1])
    TBtot = int(geom.baseTB[-1])
    RW = GPC * W

    nc = bacc.Bacc("TRN2", target_bir_lowering=False, debug=False,
                   enable_asserts=True, num_devices=NCORES)

    lhsA = nc.dram_tensor("lhsA", [K, LA], bf16, kind="ExternalInput").ap()
    rhsB = nc.dram_tensor("rhsB", [K, RW], bf16, kind="ExternalInput").ap()
    lhsB = nc.dram_tensor("lhsB", [K, LB], bf16, kind="ExternalInput").ap()
    rhsA = nc.dram_tensor("rhsA", [K, RW], bf16, kind="ExternalInput").ap()
    dirA = any(d for _, _, d in geom.groups("A"))
    dirB = any(d for _, _, d in geom.groups("B"))
    valA = nc.dram_tensor("valA", [128, TAtot], f32, kind="ExternalOutput").ap()
    valB = nc.dram_tensor("valB", [128, TBtot], f32, kind="ExternalOutput").ap()
    if dirA:
        v8A = nc.dram_tensor("v8A", [128, TAtot * 8], f32,
                             kind="ExternalOutput").ap()
    if dirB:
        v8B = nc.dram_tensor("v8B", [128, TBtot * 8], f32,
                             kind="ExternalOutput").ap()

    with tile.TileContext(nc) as tc:
        with ExitStack() as ctx:
            const = ctx.enter_context(tc.tile_pool(name="const", bufs=1))
            psum = ctx.enter_context(
                tc.tile_pool(name="psum", bufs=2, space="PSUM"))
            work = ctx.enter_context(tc.tile_pool(name="work", bufs=2))

            # Input DMAs split across both hardware queues, head chunks
            # first so the opening matmul groups can start ~3us earlier
            # than a whole-tensor transfer would allow.
            HCH = min(GSZ * 2 * 128, LA)       # lhsA head: first 2 groups
            HRW = min(4 * W, RW)               # rhsB head: first 4 slots
            lhsA_sb = const.tile([K, LA], bf16, tag="lhsA")
            nc.sync.dma_start(lhsA_sb[:, 0:HCH], lhsA[:, 0:HCH])
            rhsB_sb = const.tile([K, RW], bf16, tag="rhsB")
            nc.scalar.dma_start(rhsB_sb[:, 0:HRW], rhsB[:, 0:HRW])
            nc.sync.dma_start(lhsA_sb[:, HCH:], lhsA[:, HCH:])
            nc.scalar.dma_start(rhsB_sb[:, HRW:], rhsB[:, HRW:])
            lhsB_sb = const.tile([K, LB], bf16, tag="lhsB")
            nc.sync.dma_start(lhsB_sb[:], lhsB[:])
            rhsA_sb = const.tile([K, RW], bf16, tag="rhsA")
            nc.scalar.dma_start(rhsA_sb[:], rhsA[:])

            if NWARM:
                warm_sb = const.tile([K, PSB], bf16, tag="warm")
                nc.vector.memset(warm_sb[:], 0)
                wps = psum.tile([128, GSZ, PSB], f32, tag="ps")
                for i in range(NWARM):
                    nc.tensor.matmul(
                        wps[:, i % GSZ, :], warm_sb[:, 0:128], warm_sb[:],
                        start=True, stop=True)

            SA = const.tile([128, TAtot, W], f16, tag="SA")
            SB = const.tile([128, TBtot, W], f16, tag="SB")
            valA_sb = const.tile([128, TAtot], f32, tag="valA")
            valB_sb = const.tile([128, TBtot], f32, tag="valB")
            v8A_sb = None
            v8B_sb = None
            if dirA:
                v8A_sb = const.tile([128, TAtot * 8], f32, tag="v8A")
            if dirB:
                v8B_sb = const.tile([128, TBtot * 8], f32, tag="v8B")

            def side(side_name, lhs_sb, rhs_sb, slot_of, S, val_sb, v8_sb):
                for kk0, gn, direct in geom.groups(side_name):
                    ps = psum.tile([128, GSZ, PSB], f32, tag="ps")
                    for i in range(gn):
                        kk = kk0 + i
                        s = slot_of[kk]
                        nc.tensor.matmul(
                            ps[:, i, 0:W],
                            lhs_sb[:, kk * 128:(kk + 1) * 128],
                            rhs_sb[:, s * W:(s + 1) * W],
                            start=True, stop=True)
                    if direct:
                        for i in range(gn):
                            kk = kk0 + i
                            nc.vector.max(v8_sb[:, kk * 8:(kk + 1) * 8],
                                          ps[:, i, 0:W])
                    else:
                        nc.scalar.activation(
                            S[:, kk0:kk0 + gn, :], ps[:, 0:gn, 0:W],
                            mybir.ActivationFunctionType.Copy)
                        b1 = work.tile([128, GSZ, H1], f16, tag="b1")
                        nc.vector.tensor_tensor(
                            b1[:, 0:gn, :],
                            S[:, kk0:kk0 + gn, 0:H1],
                            S[:, kk0:kk0 + gn, H1:W], op=Alu.max)
                        b2 = work.tile([128, GSZ, H2], f16, tag="b2")
                        nc.vector.tensor_tensor(
                            b2[:, 0:gn, :],
                            b1[:, 0:gn, 0:H2],
                            b1[:, 0:gn, H2:H1], op=Alu.max)
                        nc.vector.tensor_reduce(
                            val_sb[:, kk0:kk0 + gn], b2[:, 0:gn, :],
                            Ax.X, Alu.max)

            side("A", lhsA_sb, rhsB_sb, geom.slotA, SA, valA_sb, v8A_sb)
            side("B", lhsB_sb, rhsA_sb, geom.slotB, SB, valB_sb, v8B_sb)

            nc.sync.dma_start(valA[:], valA_sb[:])
            hb = max(1, TBtot - GSZ)
            nc.sync.dma_start(valB[:, 0:hb], valB_sb[:, 0:hb])
            nc.sync.dma_start(valB[:, hb:], valB_sb[:, hb:])
            if dirA:
                nc.sync.dma_start(v8A[:], v8A_sb[:])
            if dirB:
                nc.sync.dma_start(v8B[:], v8B_sb[:])

    nc.compile()
    return nc


def _pack_side(pos_row, pos_col, starts_row, starts_col, graphs, baseT, W):
    """lhs/rhs bf16 packs for one core, one direction.

    PSUM value = 2 p.q - |q|^2 - |p|^2 = -d^2.
    K-row order: tier-2 (smallest) first, tier-0 last, |p|^2 rows last.
    """
    LT = int(baseT[-1]) * 128
    lhs = np.zeros((K, LT), dtype=np.float32)
    rhs = np.zeros((K, GPC * W), dtype=np.float32)
    #  rows 0-8   : tier2 cross (c,x3) lhs a1,a2,a3 / rhs b3,b2,b1
    #  row  9     : tier2 -q3      (lhs -1, rhs q3)
    #  rows 10-15 : tier1 cross    lhs a1,a2 / rhs b2,b1
    #  row  16    : tier1 -q2
    #  rows 17-19 : tier0 cross    lhs a1 / rhs b1
    #  row  20    : tier0 -q1  (+BIG on padding -> pad col = -BIG)
    #  rows 21-23 : -|p|^2 tiers (lhs p-squared splits, rhs -1)
    lhs[9, :] = -1.0
    lhs[16, :] = -1.0
    lhs[20, :] = -1.0
    rhs[20, :] = BIG
    tb = 0
    for s in range(GPC):
        g = graphs[s]
        p = pos_row[starts_row[g]:starts_row[g + 1]]
        n = p.shape[0]
        lb = int(baseT[s]) * 128
        for c in range(3):
            a1, a2, a3 = _split3(np.float32(2.0) * p[:, c])
            lhs[0 + c * 3, lb:lb + n] = a1
            lhs[1 + c * 3, lb:lb + n] = a2
            lhs[2 + c * 3, lb:lb + n] = a3
            lhs[10 + c * 2, lb:lb + n] = a1
            lhs[11 + c * 2, lb:lb + n] = a2
            lhs[17 + c, lb:lb + n] = a1
        pp = (p[:, 0] * p[:, 0] + p[:, 1] * p[:, 1]) + p[:, 2] * p[:, 2]
        p1, p2, p3 = _split3(pp)
        lhs[21, lb:lb + n] = p3
        lhs[22, lb:lb + n] = p2
        lhs[23, lb:lb + n] = p1

        q = pos_col[starts_col[g]:starts_col[g + 1]]
        m = q.shape[0]
        rb = s * W
        qq = (q[:, 0] * q[:, 0] + q[:, 1] * q[:, 1]) + q[:, 2] * q[:, 2]
        q1, q2, q3 = _split3(qq)
        for c in range(3):
            b1, b2, b3 = _split3(q[:, c])
            rhs[0 + c * 3, rb:rb + m] = b3
            rhs[1 + c * 3, rb:rb + m] = b2
            rhs[2 + c * 3, rb:rb + m] = b1
            rhs[10 + c * 2, rb:rb + m] = b2
            rhs[11 + c * 2, rb:rb + m] = b1
            rhs[17 + c, rb:rb + m] = b1
        rhs[9, rb:rb + m] = q3
        rhs[16, rb:rb + m] = q2
        rhs[20, rb:rb + m] = q1
        rhs[21, rb:rb + m] = -1.0
        rhs[22, rb:rb + m] = -1.0
        rhs[23, rb:rb + m] = -1.0
        tb += 1
    return lhs.astype(BF16), rhs.astype(BF16)


def _unpack_side(val, v8, groups, starts_row, graphs, TA, baseT, dist_full):
    direct_kks = set()
    for kk0, gn, direct in groups:
        if direct:
            direct_kks.update(range(kk0, kk0 + gn))
    for s in range(GPC):
        g = graphs[s]
        n = starts_row[g + 1] - starts_row[g]
        for t in range((n + 127) // 128):
            rows = min(128, n - t * 128)
            kk = int(baseT[s]) + t
            if kk in direct_kks:
                v = v8[:rows, kk * 8].astype(np.float64)
            else:
                v = val[:rows, kk].astype(np.float64)
            atoms = starts_row[g] + t * 128 + np.arange(rows)
            dist_full[atoms] = np.sqrt(np.maximum(-v, 0.0))


def _patch_rows(pos_row, pos_col, starts_row, starts_col, n2g_row,
                dist_full):
    """Exact recompute (reference arithmetic) for rows near the cutoff."""
    sel = np.nonzero(np.abs(dist_full - 10.0) < PATCH_BAND)[0]
    for r in sel:
        g = int(n2g_row[r])
        Q = pos_col[starts_col[g]:starts_col[g + 1]]
        p = pos_row[r]
        d2 = ((p[0] - Q[:, 0]) ** 2 + (p[1] - Q[:, 1]) ** 2
              + (p[2] - Q[:, 2]) ** 2)
        j = int(np.argmin(d2))
        d = p - Q[j]
        dist_full[r] = np.sqrt((d[0] * d[0] + d[1] * d[1]) + d[2] * d[2])


def kernel(pos_a, pos_b, node2graph_a, node2graph_b,
           atom2residue_a, atom2residue_b, is_mutation):
    global LAST_EXEC_NS

    from concourse.bass_utils import run_bass_kernel_spmd

    pos_a = np.asarray(pos_a, dtype=np.float32)
    pos_b = np.asarray(pos_b, dtype=np.float32)
    node2graph_a = np.asarray(node2graph_a)
    node2graph_b = np.asarray(node2graph_b)
    atom2residue_a = np.asarray(atom2residue_a)
    atom2residue_b = np.asarray(atom2residue_b)
    is_mutation = np.asarray(is_mutation)

    Na = pos_a.shape[0]
    Nb = pos_b.shape[0]

    sa = np.searchsorted(node2graph_a, np.arange(G + 1)).astype(np.int64)
    sb = np.searchsorted(node2graph_b, np.arange(G + 1)).astype(np.int64)
    na = np.diff(sa)
    nb = np.diff(sb)
    assert na.min() > 0 and nb.min() > 0, "empty graph block not supported"

    geom = _Geom(na, nb)
    key = geom.key()
    if key not in _prog_cache:
        _prog_cache[key] = _build_program(geom)
    nc = _prog_cache[key]

    in_maps = []
    for c in range(NCORES):
        lhsA, rhsB = _pack_side(pos_a, pos_b, sa, sb, geom.graphA[c],
                                geom.baseTA, geom.WSTAR)
        lhsB, rhsA = _pack_side(pos_b, pos_a, sb, sa, geom.graphB[c],
                                geom.baseTB, geom.WSTAR)
        in_maps.append({"lhsA": lhsA, "rhsB": rhsB,
                        "lhsB": lhsB, "rhsA": rhsA})

    if PROFILE:
        _install_ntff_hook()
    res = run_bass_kernel_spmd(nc, in_maps, list(range(NCORES)),
                               trace=bool(PROFILE))
    if PROFILE:
        LAST_EXEC_NS = res.exec_time_ns

    dist_a = np.zeros(Na, dtype=np.float64)
    dist_b = np.zeros(Nb, dtype=np.float64)
    gA = geom.groups("A")
    gB = geom.groups("B")
    for c in range(NCORES):
        _unpack_side(res.results[c]["valA"], res.results[c].get("v8A"), gA,
                     sa, geom.graphA[c], geom.TA, geom.baseTA, dist_a)
        _unpack_side(res.results[c]["valB"], res.results[c].get("v8B"), gB,
                     sb, geom.graphB[c], geom.TB, geom.baseTB, dist_b)

    dist_a = dist_a.astype(np.float32)
    dist_b = dist_b.astype(np.float32)
    _patch_rows(pos_a, pos_b, sa, sb, node2graph_a, dist_a)
    _patch_rows(pos_b, pos_a, sb, sa, node2graph_b, dist_b)

    def iface_mask(dist, atom2residue):
        is_if = (dist < CUTOFF).astype(np.int32)
        res_max = np.zeros(NUM_RESIDUES, dtype=np.int32)
        np.maximum.at(res_max, atom2residue, is_if)
        return res_max[atom2residue] > 0

    mask_a = iface_mask(dist_a, atom2residue_a)
    mask_b = iface_mask(dist_b, atom2residue_b)
    mask = np.concatenate([mask_a, mask_b]) | is_mutation.astype(bool)
    dists = np.concatenate([dist_a, dist_b]).astype(np.float32)
    return mask, dists


# revision 27
# speedup vs baseline: 1.0514x; 1.0190x over previous
"""Trainium2 Bass kernel for nn_InterfaceGraph (retrieval_knn).

Segment-restricted nearest-neighbor DISTANCES between pos_a and pos_b
(16384 x 16384 pairwise, block-diagonal over 64 sorted graphs), sharded
over 8 NeuronCores (8 graphs per core).

Key observation: the reference returns only (mask, dists) -- no
indices.  dist = sqrt(min d^2), and min d^2 = -max(2 a.b - |b|^2 -
|a|^2), so the row-max VALUE of the K=24 bf16-split matmul is the
answer; no argmax index extraction is needed at all.

Per 128-row tile, the matmul writes -d^2 into PSUM (f32), four tiles
(four PSUM banks) per group, double-buffered.  The otherwise-idle
scalar engine drains each group to SBUF fp16 in one wide activation
copy; the vector engine reduces with two fp16 tensor_tensor max folds
(DVE 2x packed mode) plus a tensor_reduce over the 74-wide tail.
The three engines pipeline at ~1.24 us per 4-tile group, co-bound.
Graphs are assigned to cores by a deterministic swap-search that
minimizes the total row-tile count on both sides at once (40 tiles =
10 groups per core).  Input DMAs are split head/tail across both
hardware queues so the first groups start ~3 us earlier.

The PE HAM clock gate stays at K=4/8 (1.2 GHz) for kernels this
short on this part (verified with an 8.6 us dummy-matmul burst), so
matmuls stream at ~0.83 ns/col; a ~10 us semaphore-drain teardown and
~4 us DMA/preamble startup are fixed harness costs in the measured
exec time.

fp16 rounding gives ~5e-4 relative dist error (tolerance 2e-2).
Atoms whose estimated dist lies within 0.05 of the 10.0 interface
cutoff are recomputed exactly on host (a few hundred rows), which
also makes the cutoff mask bit-exact.
"""

import numpy as np
import ml_dtypes

NCORES = 8
G = 64
GPC = G // NCORES
NUM_RESIDUES = 2048
CUTOFF = np.float32(10.0)
BIG = np.float32(2.0 ** 26)
K = 24            # 18 cross rows + 3 |b|^2 rows + 3 |a|^2 rows
GSZ = 4           # tiles per psum group, one 2KB psum bank per tile
PSB = 512         # f32 per psum bank (matmul output must not cross banks)
PATCH_BAND = 0.05
DIRECT_FRAC = 0.0    # fraction of tile-groups reduced straight from PSUM
NWARM = 0         # dummy matmuls before the real stream (HAM never opens
                  # on this part -- K stays 4/8 -- so warmup only delays)

PROFILE = False
LAST_EXEC_NS = None

BF16 = ml_dtypes.bfloat16

_prog_cache = {}


def _round_up(x, m):
    return (x + m - 1) // m * m


def _install_ntff_hook():
    import sys
    import types
    if 'antenv.axon_hooks' in sys.modules:
        return
    from trn_agent_boot.trn_boot import _ntff_profile_via_ctypes
    hook = _ntff_profile_via_ctypes('/opt/axon/libaxon_pjrt.so')
    mod = types.ModuleType('antenv.axon_hooks')
    mod.get_axon_ntff_profile_hook = lambda: hook
    sys.modules['antenv.axon_hooks'] = mod


def _split3(v):
    """bf16x3 split: v ~= v1 + v2 + v3 with ~24-bit mantissa coverage."""
    v = v.astype(np.float32)
    v1 = v.astype(BF16).astype(np.float32)
    r = v - v1
    v2 = r.astype(BF16).astype(np.float32)
    v3 = (r - v2).astype(BF16).astype(np.float32)
    return v1, v2, v3


class _Geom:
    """Per-slot shapes shared by all cores (SPMD program is one program).

    Graphs are assigned to cores by _assign, then slot-sorted per side
    (by row count) so per-slot cross-core tile maxima stay tight.  The
    reduction window is a single uniform width WSTAR for every slot,
    which lets each PSUM group hold GSZ bank-aligned matmul outputs and
    the fold tree run as a few wide strided ops per group.
    """

    def __init__(self, na, nb):
        assign = self._assign(na, nb)          # [core, i] -> graph id
        self.graphA = np.zeros((NCORES, GPC), dtype=np.int64)
        self.graphB = np.zeros((NCORES, GPC), dtype=np.int64)
        for c in range(NCORES):
            gs = assign[c]
            self.graphA[c] = gs[np.argsort(-na[gs], kind="stable")]
            self.graphB[c] = gs[np.argsort(-nb[gs], kind="stable")]
        na_A = na[self.graphA]
        nb_B = nb[self.graphB]
        self.TA = [int(-(-na_A[:, s].max() // 128)) for s in range(GPC)]
        self.TB = [int(-(-nb_B[:, s].max() // 128)) for s in range(GPC)]
        self.baseTA = np.concatenate([[0], np.cumsum(self.TA)]).astype(int)
        self.baseTB = np.concatenate([[0], np.cumsum(self.TB)]).astype(int)
        # uniform fold window: multiple of 8 so 3 halvings stay integral
        self.WSTAR = int(_round_up(int(max(na.max(), nb.max())), 8))
        # slot id for each tile index
        self.slotA = sum(([s] * self.TA[s] for s in range(GPC)), [])
        self.slotB = sum(([s] * self.TB[s] for s in range(GPC)), [])

    @staticmethod
    def _assign(na, nb):
        """Deterministic graph->core assignment minimizing total row-tile
        count (the per-slot cross-core maxima on both sides)."""
        def cost(a):
            A = np.sort(na[a], axis=1)[:, ::-1]
            B = np.sort(nb[a], axis=1)[:, ::-1]
            ta = sum(int(-(-A[:, s].max() // 128)) for s in range(GPC))
            tb = sum(int(-(-B[:, s].max() // 128)) for s in range(GPC))
            return ta + tb
        order = np.argsort(-na, kind="stable")
        best = np.zeros((NCORES, GPC), dtype=np.int64)
        for r, g in enumerate(order):        # na-balanced snake deal
            c = r % NCORES if (r // NCORES) % 2 == 0 else \
                NCORES - 1 - r % NCORES
            best[c, r // NCORES] = g
        bcost = cost(best)
        rng = np.random.default_rng(0)
        for _ in range(8000):                # swap search (deterministic)
            c1, c2 = rng.integers(0, NCORES, 2)
            i1, i2 = rng.integers(0, GPC, 2)
            a = best.copy()
            a[c1, i1], a[c2, i2] = a[c2, i2], a[c1, i1]
            ac = cost(a)
            if ac <= bcost:
                best, bcost = a, ac
        return best

    def groups(self, side):
        """[(kk0, gn, direct)] covering all tiles of a side."""
        tot = int(self.baseTA[-1] if side == "A" else self.baseTB[-1])
        out = []
        kk0 = 0
        while kk0 < tot:
            gn = min(GSZ, tot - kk0)
            out.append([kk0, gn, False])
            kk0 += gn
        ndirect = max(0, int(round(DIRECT_FRAC * len(out))))
        for i in range(len(out) - ndirect, len(out)):
            out[i][2] = True
        return [tuple(g) for g in out]

    def key(self):
        return (tuple(self.TA), tuple(self.TB), self.WSTAR)


def _build_program(geom):
    from contextlib import ExitStack

    import concourse.bacc as bacc
    import concourse.mybir as mybir
    import concourse.tile as tile

    f32 = mybir.dt.float32
    f16 = mybir.dt.float16
    bf16 = mybir.dt.bfloat16
    Alu = mybir.AluOpType
    Ax = mybir.AxisListType

    W = geom.WSTAR
    H1, H2, H3 = W // 2, W // 4, W // 8
    LA = int(geom.baseTA[-1]) * 128
    LB = int(geom.baseTB[-1]) * 128
    TAtot = int(geom.baseTA[-1])
    TBtot = int(geom.baseTB[-1])
    RW = GPC * W

    nc = bacc.Bacc("TRN2", target_bir_lowering=False, debug=False,
                   enable_asserts=True, num_devices=NCORES)

    lhsA = nc.dram_tensor("lhsA", [K, LA], bf16, kind="ExternalInput").ap()
    rhsB = nc.dram_tensor("rhsB", [K, RW], bf16, kind="ExternalInput").ap()
    lhsB = nc.dram_tensor("lhsB", [K, LB], bf16, kind="ExternalInput").ap()
    rhsA = nc.dram_tensor("rhsA", [K, RW], bf16, kind="ExternalInput").ap()
    dirA = any(d for _, _, d in geom.groups("A"))
    dirB = any(d for _, _, d in geom.groups("B"))
    valA = nc.dram_tensor("valA", [128, TAtot], f32, kind="ExternalOutput").ap()
    valB = nc.dram_tensor("valB", [128, TBtot], f32, kind="ExternalOutput").ap()
    if dirA:
        v8A = nc.dram_tensor("v8A", [128, TAtot * 8], f32,
                             kind="ExternalOutput").ap()
    if dirB:
        v8B = nc.dram_tensor("v8B", [128, TBtot * 8], f32,
                             kind="ExternalOutput").ap()

    with tile.TileContext(nc) as tc:
        with ExitStack() as ctx:
            const = ctx.enter_context(tc.tile_pool(name="const", bufs=1))
            psum = ctx.enter_context(
                tc.tile_pool(name="psum", bufs=2, space="PSUM"))
            work = ctx.enter_context(tc.tile_pool(name="work", bufs=2))

            # Input DMAs split across both hardware queues, head chunks
            # first so the opening matmul groups can start ~3us earlier
            # than a whole-tensor transfer would allow.
            HCH = min(GSZ * 2 * 128, LA)       # lhsA head: first 2 groups
            HRW = min(4 * W, RW)               # rhsB head: first 4 slots
            lhsA_sb = const.tile([K, LA], bf16, tag="lhsA")
            nc.sync.dma_start(lhsA_sb[:, 0:HCH], lhsA[:, 0:HCH])
            rhsB_sb = const.tile([K, RW], bf16, tag="rhsB")
            nc.scalar.dma_start(rhsB_sb[:, 0:HRW], rhsB[:, 0:HRW])
            nc.sync.dma_start(lhsA_sb[:, HCH:], lhsA[:, HCH:])
            nc.scalar.dma_start(rhsB_sb[:, HRW:], rhsB[:, HRW:])
            lhsB_sb = const.tile([K, LB], bf16, tag="lhsB")
            nc.sync.dma_start(lhsB_sb[:], lhsB[:])
            rhsA_sb = const.tile([K, RW], bf16, tag="rhsA")
            nc.scalar.dma_start(rhsA_sb[:], rhsA[:])

            if NWARM:
                warm_sb = const.tile([K, PSB], bf16, tag="warm")
                nc.vector.memset(warm_sb[:], 0)
                wps = psum.tile([128, GSZ, PSB], f32, tag="ps")
                for i in range(NWARM):
                    nc.tensor.matmul(
                        wps[:, i % GSZ, :], warm_sb[:, 0:128], warm_sb[:],
                        start=True, stop=True)

            SA = const.tile([128, TAtot, W], f16, tag="SA")
            SB = const.tile([128, TBtot, W], f16, tag="SB")
            valA_sb = const.tile([128, TAtot], f32, tag="valA")
            valB_sb = const.tile([128, TBtot], f32, tag="valB")
            v8A_sb = None
            v8B_sb = None
            if dirA:
                v8A_sb = const.tile([128, TAtot * 8], f32, tag="v8A")
            if dirB:
                v8B_sb = const.tile([128, TBtot * 8], f32, tag="v8B")

            def side(side_name, lhs_sb, rhs_sb, slot_of, S, val_sb, v8_sb):
                for kk0, gn, direct in geom.groups(side_name):
                    ps = psum.tile([128, GSZ, PSB], f32, tag="ps")
                    for i in range(gn):
                        kk = kk0 + i
                        s = slot_of[kk]
                        nc.tensor.matmul(
                            ps[:, i, 0:W],
                            lhs_sb[:, kk * 128:(kk + 1) * 128],
                            rhs_sb[:, s * W:(s + 1) * W],
                            start=True, stop=True)
                    if direct:
                        for i in range(gn):
                            kk = kk0 + i
                            nc.vector.max(v8_sb[:, kk * 8:(kk + 1) * 8],
                                          ps[:, i, 0:W])
                    else:
                        nc.scalar.activation(
                            S[:, kk0:kk0 + gn, :], ps[:, 0:gn, 0:W],
                            mybir.ActivationFunctionType.Copy)
                        b1 = work.tile([128, GSZ, H1], f16, tag="b1")
                        nc.vector.tensor_tensor(
                            b1[:, 0:gn, :],
                            S[:, kk0:kk0 + gn, 0:H1],
                            S[:, kk0:kk0 + gn, H1:W], op=Alu.max)
                        b2 = work.tile([128, GSZ, H2], f16, tag="b2")
                        nc.vector.tensor_tensor(
                            b2[:, 0:gn, :],
                            b1[:, 0:gn, 0:H2],
                            b1[:, 0:gn, H2:H1], op=Alu.max)
                        nc.vector.tensor_reduce(
                            val_sb[:, kk0:kk0 + gn], b2[:, 0:gn, :],
                            Ax.X, Alu.max)

            side("A", lhsA_sb, rhsB_sb, geom.slotA, SA, valA_sb, v8A_sb)
            side("B", lhsB_sb, rhsA_sb, geom.slotB, SB, valB_sb, v8B_sb)

            nc.sync.dma_start(valA[:], valA_sb[:])
            hb = max(1, TBtot - GSZ)
            nc.sync.dma_start(valB[:, 0:hb], valB_sb[:, 0:hb])
            nc.sync.dma_start(valB[:, hb:], valB_sb[:, hb:])
            if dirA:
                nc.sync.dma_start(v8A[:], v8A_sb[:])
            if dirB:
                nc.sync.dma_start(v8B[:], v8B_sb[:])

    nc.compile()
    return nc


def _pack_side(pos_row, pos_col, starts_row, starts_col, graphs, baseT, W):
    """lhs/rhs bf16 packs for one core, one direction.

    PSUM value = 2 p.q - |q|^2 - |p|^2 = -d^2.
    K-row order: tier-2 (smallest) first, tier-0 last, |p|^2 rows last.
    """
    LT = int(baseT[-1]) * 128
    lhs = np.zeros((K, LT), dtype=np.float32)
    rhs = np.zeros((K, GPC * W), dtype=np.float32)
    #  rows 0-8   : tier2 cross (c,x3) lhs a1,a2,a3 / rhs b3,b2,b1
    #  row  9     : tier2 -q3      (lhs -1, rhs q3)
    #  rows 10-15 : tier1 cross    lhs a1,a2 / rhs b2,b1
    #  row  16    : tier1 -q2
    #  rows 17-19 : tier0 cross    lhs a1 / rhs b1
    #  row  20    : tier0 -q1  (+BIG on padding -> pad col = -BIG)
    #  rows 21-23 : -|p|^2 tiers (lhs p-squared splits, rhs -1)
    lhs[9, :] = -1.0
    lhs[16, :] = -1.0
    lhs[20, :] = -1.0
    rhs[20, :] = BIG
    tb = 0
    for s in range(GPC):
        g = graphs[s]
        p = pos_row[starts_row[g]:starts_row[g + 1]]
        n = p.shape[0]
        lb = int(baseT[s]) * 128
        for c in range(3):
            a1, a2, a3 = _split3(np.float32(2.0) * p[:, c])
            lhs[0 + c * 3, lb:lb + n] = a1
            lhs[1 + c * 3, lb:lb + n] = a2
            lhs[2 + c * 3, lb:lb + n] = a3
            lhs[10 + c * 2, lb:lb + n] = a1
            lhs[11 + c * 2, lb:lb + n] = a2
            lhs[17 + c, lb:lb + n] = a1
        pp = (p[:, 0] * p[:, 0] + p[:, 1] * p[:, 1]) + p[:, 2] * p[:, 2]
        p1, p2, p3 = _split3(pp)
        lhs[21, lb:lb + n] = p3
        lhs[22, lb:lb + n] = p2
        lhs[23, lb:lb + n] = p1

        q = pos_col[starts_col[g]:starts_col[g + 1]]
        m = q.shape[0]
        rb = s * W
        qq = (q[:, 0] * q[:, 0] + q[:, 1] * q[:, 1]) + q[:, 2] * q[:, 2]
        q1, q2, q3 = _split3(qq)
        for c in range(3):
            b1, b2, b3 = _split3(q[:, c])
            rhs[0 + c * 3, rb:rb + m] = b3
            rhs[1 + c * 3, rb:rb + m] = b2
            rhs[2 + c * 3, rb:rb + m] = b1
            rhs[10 + c * 2, rb:rb + m] = b2
            rhs[11 + c * 2, rb:rb + m] = b1
            rhs[17 + c, rb:rb + m] = b1
        rhs[9, rb:rb + m] = q3
        rhs[16, rb:rb + m] = q2
        rhs[20, rb:rb + m] = q1
        rhs[21, rb:rb + m] = -1.0
        rhs[22, rb:rb + m] = -1.0
        rhs[23, rb:rb + m] = -1.0
        tb += 1
    return lhs.astype(BF16), rhs.astype(BF16)


def _unpack_side(val, v8, groups, starts_row, graphs, TA, baseT, dist_full):
    direct_kks = set()
    for kk0, gn, direct in groups:
        if direct:
            direct_kks.update(range(kk0, kk0 + gn))
    for s in range(GPC):
        g = graphs[s]
        n = starts_row[g + 1] - starts_row[g]
        for t in range((n + 127) // 128):
            rows = min(128, n - t * 128)
            kk = int(baseT[s]) + t
            if kk in direct_kks:
                v = v8[:rows, kk * 8].astype(np.float64)
            else:
                v = val[:rows, kk].astype(np.float64)
            atoms = starts_row[g] + t * 128 + np.arange(rows)
            dist_full[atoms] = np.sqrt(np.maximum(-v, 0.0))


def _patch_rows(pos_row, pos_col, starts_row, starts_col, n2g_row,
                dist_full):
    """Exact recompute (reference arithmetic) for rows near the cutoff."""
    sel = np.nonzero(np.abs(dist_full - 10.0) < PATCH_BAND)[0]
    for r in sel:
        g = int(n2g_row[r])
        Q = pos_col[starts_col[g]:starts_col[g + 1]]
        p = pos_row[r]
        d2 = ((p[0] - Q[:, 0]) ** 2 + (p[1] - Q[:, 1]) ** 2
              + (p[2] - Q[:, 2]) ** 2)
        j = int(np.argmin(d2))
        d = p - Q[j]
        dist_full[r] = np.sqrt((d[0] * d[0] + d[1] * d[1]) + d[2] * d[2])


def kernel(pos_a, pos_b, node2graph_a, node2graph_b,
           atom2residue_a, atom2residue_b, is_mutation):
    global LAST_EXEC_NS

    from concourse.bass_utils import run_bass_kernel_spmd

    pos_a = np.asarray(pos_a, dtype=np.float32)
    pos_b = np.asarray(pos_b, dtype=np.float32)
    node2graph_a = np.asarray(node2graph_a)
    node2graph_b = np.asarray(node2graph_b)
    atom2residue_a = np.asarray(atom2residue_a)
    atom2residue_b = np.asarray(atom2residue_b)
    is_mutation = np.asarray(is_mutation)

    Na = pos_a.shape[0]
    Nb = pos_b.shape[0]

    sa = np.searchsorted(node2graph_a, np.arange(G + 1)).astype(np.int64)
    sb = np.searchsorted(node2graph_b, np.arange(G + 1)).astype(np.int64)
    na = np.diff(sa)
    nb = np.diff(sb)
    assert na.min() > 0 and nb.min() > 0, "empty graph block not supported"

    geom = _Geom(na, nb)
    key = geom.key()
    if key not in _prog_cache:
        _prog_cache[key] = _build_program(geom)
    nc = _prog_cache[key]

    in_maps = []
    for c in range(NCORES):
        lhsA, rhsB = _pack_side(pos_a, pos_b, sa, sb, geom.graphA[c],
                                geom.baseTA, geom.WSTAR)
        lhsB, rhsA = _pack_side(pos_b, pos_a, sb, sa, geom.graphB[c],
                                geom.baseTB, geom.WSTAR)
        in_maps.append({"lhsA": lhsA, "rhsB": rhsB,
                        "lhsB": lhsB, "rhsA": rhsA})

    if PROFILE:
        _install_ntff_hook()
    res = run_bass_kernel_spmd(nc, in_maps, list(range(NCORES)),
                               trace=bool(PROFILE))
    if PROFILE:
        LAST_EXEC_NS = res.exec_time_ns

    dist_a = np.zeros(Na, dtype=np.float64)
    dist_b = np.zeros(Nb, dtype=np.float64)
    gA = geom.groups("A")
    gB = geom.groups("B")
    for c in range(NCORES):
        _unpack_side(res.results[c]["valA"], res.results[c].get("v8A"), gA,
                     sa, geom.graphA[c], geom.TA, geom.baseTA, dist_a)
        _unpack_side(res.results[c]["valB"], res.results[c].get("v8B"), gB,
                     sb, geom.graphB[c], geom.TB, geom.baseTB, dist_b)

    dist_a = dist_a.astype(np.float32)
    dist_b = dist_b.astype(np.float32)
    _patch_rows(pos_a, pos_b, sa, sb, node2graph_a, dist_a)
    _patch_rows(pos_b, pos_a, sb, sa, node2graph_b, dist_b)

    def iface_mask(dist, atom2residue):
        is_if = (dist < CUTOFF).astype(np.int32)
        res_max = np.zeros(NUM_RESIDUES, dtype=np.int32)
        np.maximum.at(res_max, atom2residue, is_if)
        return res_max[atom2residue] > 0

    mask_a = iface_mask(dist_a, atom2residue_a)
    mask_b = iface_mask(dist_b, atom2residue_b)
    mask = np.concatenate([mask_a, mask_b]) | is_mutation.astype(bool)
    dists = np.concatenate([dist_a, dist_b]).astype(np.float32)
    return mask, dists


# revision 28
# speedup vs baseline: 1.0524x; 1.0010x over previous
"""Trainium2 Bass kernel for nn_InterfaceGraph (retrieval_knn).

Segment-restricted nearest-neighbor DISTANCES between pos_a and pos_b
(16384 x 16384 pairwise, block-diagonal over 64 sorted graphs), sharded
over 8 NeuronCores (8 graphs per core).

Key observation: the reference returns only (mask, dists) -- no
indices.  dist = sqrt(min d^2), and min d^2 = -max(2 a.b - |b|^2 -
|a|^2), so the row-max VALUE of the K=24 bf16-split matmul is the
answer; no argmax index extraction is needed at all.

Per 128-row tile, the matmul writes -d^2 into PSUM (f32), four tiles
(four PSUM banks) per group, double-buffered.  The otherwise-idle
scalar engine drains each group to SBUF fp16 in one wide activation
copy; the vector engine reduces with two fp16 tensor_tensor max folds
(DVE 2x packed mode) plus a tensor_reduce over the 74-wide tail.
The three engines pipeline at ~1.24 us per 4-tile group, co-bound.
Graphs are assigned to cores by a deterministic swap-search that
minimizes the total row-tile count on both sides at once (40 tiles =
10 groups per core).  Input DMAs are split head/tail across both
hardware queues so the first groups start ~3 us earlier.

The PE HAM clock gate stays at K=4/8 (1.2 GHz) for kernels this
short on this part (verified with an 8.6 us dummy-matmul burst), so
matmuls stream at ~0.83 ns/col; a ~10 us semaphore-drain teardown and
~4 us DMA/preamble startup are fixed harness costs in the measured
exec time.

fp16 rounding gives ~5e-4 relative dist error (tolerance 2e-2).
Atoms whose estimated dist lies within 0.05 of the 10.0 interface
cutoff are recomputed exactly on host (a few hundred rows), which
also makes the cutoff mask bit-exact.
"""

import numpy as np
import ml_dtypes

NCORES = 8
G = 64
GPC = G // NCORES
NUM_RESIDUES = 2048
CUTOFF = np.float32(10.0)
BIG = np.float32(2.0 ** 26)
K = 24            # 18 cross rows + 3 |b|^2 rows + 3 |a|^2 rows
GSZ = 4           # tiles per psum group, one 2KB psum bank per tile
PSB = 512         # f32 per psum bank (matmul output must not cross banks)
PATCH_BAND = 0.05
DIRECT_FRAC = 0.0    # fraction of tile-groups reduced straight from PSUM
NWARM = 0         # dummy matmuls before the real stream (HAM never opens
                  # on this part -- K stays 4/8 -- so warmup only delays)

PROFILE = False
LAST_EXEC_NS = None

BF16 = ml_dtypes.bfloat16

_prog_cache = {}


def _round_up(x, m):
    return (x + m - 1) // m * m


def _install_ntff_hook():
    import sys
    import types
    if 'antenv.axon_hooks' in sys.modules:
        return
    from trn_agent_boot.trn_boot import _ntff_profile_via_ctypes
    hook = _ntff_profile_via_ctypes('/opt/axon/libaxon_pjrt.so')
    mod = types.ModuleType('antenv.axon_hooks')
    mod.get_axon_ntff_profile_hook = lambda: hook
    sys.modules['antenv.axon_hooks'] = mod


def _split3(v):
    """bf16x3 split: v ~= v1 + v2 + v3 with ~24-bit mantissa coverage."""
    v = v.astype(np.float32)
    v1 = v.astype(BF16).astype(np.float32)
    r = v - v1
    v2 = r.astype(BF16).astype(np.float32)
    v3 = (r - v2).astype(BF16).astype(np.float32)
    return v1, v2, v3


class _Geom:
    """Per-slot shapes shared by all cores (SPMD program is one program).

    Graphs are assigned to cores by _assign, then slot-sorted per side
    (by row count) so per-slot cross-core tile maxima stay tight.  The
    reduction window is a single uniform width WSTAR for every slot,
    which lets each PSUM group hold GSZ bank-aligned matmul outputs and
    the fold tree run as a few wide strided ops per group.
    """

    def __init__(self, na, nb):
        assign = self._assign(na, nb)          # [core, i] -> graph id
        self.graphA = np.zeros((NCORES, GPC), dtype=np.int64)
        self.graphB = np.zeros((NCORES, GPC), dtype=np.int64)
        for c in range(NCORES):
            gs = assign[c]
            self.graphA[c] = gs[np.argsort(-na[gs], kind="stable")]
            self.graphB[c] = gs[np.argsort(-nb[gs], kind="stable")]
        na_A = na[self.graphA]
        nb_B = nb[self.graphB]
        self.TA = [int(-(-na_A[:, s].max() // 128)) for s in range(GPC)]
        self.TB = [int(-(-nb_B[:, s].max() // 128)) for s in range(GPC)]
        self.baseTA = np.concatenate([[0], np.cumsum(self.TA)]).astype(int)
        self.baseTB = np.concatenate([[0], np.cumsum(self.TB)]).astype(int)
        # uniform fold window: multiple of 8 so 3 halvings stay integral
        self.WSTAR = int(_round_up(int(max(na.max(), nb.max())), 8))
        # slot id for each tile index
        self.slotA = sum(([s] * self.TA[s] for s in range(GPC)), [])
        self.slotB = sum(([s] * self.TB[s] for s in range(GPC)), [])

    @staticmethod
    def _assign(na, nb):
        """Deterministic graph->core assignment minimizing total row-tile
        count (the per-slot cross-core maxima on both sides)."""
        def cost(a):
            A = np.sort(na[a], axis=1)[:, ::-1]
            B = np.sort(nb[a], axis=1)[:, ::-1]
            ta = sum(int(-(-A[:, s].max() // 128)) for s in range(GPC))
            tb = sum(int(-(-B[:, s].max() // 128)) for s in range(GPC))
            return ta + tb
        order = np.argsort(-na, kind="stable")
        best = np.zeros((NCORES, GPC), dtype=np.int64)
        for r, g in enumerate(order):        # na-balanced snake deal
            c = r % NCORES if (r // NCORES) % 2 == 0 else \
                NCORES - 1 - r % NCORES
            best[c, r // NCORES] = g
        bcost = cost(best)
        rng = np.random.default_rng(0)
        for _ in range(8000):                # swap search (deterministic)
            c1, c2 = rng.integers(0, NCORES, 2)
            i1, i2 = rng.integers(0, GPC, 2)
            a = best.copy()
            a[c1, i1], a[c2, i2] = a[c2, i2], a[c1, i1]
            ac = cost(a)
            if ac <= bcost:
                best, bcost = a, ac
        return best

    def groups(self, side):
        """[(kk0, gn, direct)] covering all tiles of a side."""
        tot = int(self.baseTA[-1] if side == "A" else self.baseTB[-1])
        out = []
        kk0 = 0
        while kk0 < tot:
            gn = min(GSZ, tot - kk0)
            out.append([kk0, gn, False])
            kk0 += gn
        # Side B ends the whole stream: split its last group so the
        # trailing copy + fold chain covers a single tile (~1.2 us less
        # serial tail after the final matmul).
        if side == "B" and out[-1][1] > 1:
            k0, gn, d = out[-1]
            out[-1] = [k0, gn - 1, d]
            out.append([k0 + gn - 1, 1, d])
        ndirect = max(0, int(round(DIRECT_FRAC * len(out))))
        for i in range(len(out) - ndirect, len(out)):
            out[i][2] = True
        return [tuple(g) for g in out]

    def key(self):
        return (tuple(self.TA), tuple(self.TB), self.WSTAR)


def _build_program(geom):
    from contextlib import ExitStack

    import concourse.bacc as bacc
    import concourse.mybir as mybir
    import concourse.tile as tile

    f32 = mybir.dt.float32
    f16 = mybir.dt.float16
    bf16 = mybir.dt.bfloat16
    Alu = mybir.AluOpType
    Ax = mybir.AxisListType

    W = geom.WSTAR
    H1, H2, H3 = W // 2, W // 4, W // 8
    LA = int(geom.baseTA[-1]) * 128
    LB = int(geom.baseTB[-1]) * 128
    TAtot = int(geom.baseTA[-1])
    TBtot = int(geom.baseTB[-1])
    RW = GPC * W

    nc = bacc.Bacc("TRN2", target_bir_lowering=False, debug=False,
                   enable_asserts=True, num_devices=NCORES)

    lhsA = nc.dram_tensor("lhsA", [K, LA], bf16, kind="ExternalInput").ap()
    rhsB = nc.dram_tensor("rhsB", [K, RW], bf16, kind="ExternalInput").ap()
    lhsB = nc.dram_tensor("lhsB", [K, LB], bf16, kind="ExternalInput").ap()
    rhsA = nc.dram_tensor("rhsA", [K, RW], bf16, kind="ExternalInput").ap()
    dirA = any(d for _, _, d in geom.groups("A"))
    dirB = any(d for _, _, d in geom.groups("B"))
    valA = nc.dram_tensor("valA", [128, TAtot], f32, kind="ExternalOutput").ap()
    valB = nc.dram_tensor("valB", [128, TBtot], f32, kind="ExternalOutput").ap()
    if dirA:
        v8A = nc.dram_tensor("v8A", [128, TAtot * 8], f32,
                             kind="ExternalOutput").ap()
    if dirB:
        v8B = nc.dram_tensor("v8B", [128, TBtot * 8], f32,
                             kind="ExternalOutput").ap()

    with tile.TileContext(nc) as tc:
        with ExitStack() as ctx:
            const = ctx.enter_context(tc.tile_pool(name="const", bufs=1))
            psum = ctx.enter_context(
                tc.tile_pool(name="psum", bufs=2, space="PSUM"))
            work = ctx.enter_context(tc.tile_pool(name="work", bufs=2))

            # Input DMAs split across both hardware queues, head chunks
            # first so the opening matmul groups can start ~3us earlier
            # than a whole-tensor transfer would allow.
            HCH = min(GSZ * 2 * 128, LA)       # lhsA head: first 2 groups
            HRW = min(4 * W, RW)               # rhsB head: first 4 slots
            lhsA_sb = const.tile([K, LA], bf16, tag="lhsA")
            nc.sync.dma_start(lhsA_sb[:, 0:HCH], lhsA[:, 0:HCH])
            rhsB_sb = const.tile([K, RW], bf16, tag="rhsB")
            nc.scalar.dma_start(rhsB_sb[:, 0:HRW], rhsB[:, 0:HRW])
            nc.sync.dma_start(lhsA_sb[:, HCH:], lhsA[:, HCH:])
            nc.scalar.dma_start(rhsB_sb[:, HRW:], rhsB[:, HRW:])
            lhsB_sb = const.tile([K, LB], bf16, tag="lhsB")
            nc.sync.dma_start(lhsB_sb[:], lhsB[:])
            rhsA_sb = const.tile([K, RW], bf16, tag="rhsA")
            nc.scalar.dma_start(rhsA_sb[:], rhsA[:])

            if NWARM:
                warm_sb = const.tile([K, PSB], bf16, tag="warm")
                nc.vector.memset(warm_sb[:], 0)
                wps = psum.tile([128, GSZ, PSB], f32, tag="ps")
                for i in range(NWARM):
                    nc.tensor.matmul(
                        wps[:, i % GSZ, :], warm_sb[:, 0:128], warm_sb[:],
                        start=True, stop=True)

            SA = const.tile([128, TAtot, W], f16, tag="SA")
            SB = const.tile([128, TBtot, W], f16, tag="SB")
            valA_sb = const.tile([128, TAtot], f32, tag="valA")
            valB_sb = const.tile([128, TBtot], f32, tag="valB")
            v8A_sb = None
            v8B_sb = None
            if dirA:
                v8A_sb = const.tile([128, TAtot * 8], f32, tag="v8A")
            if dirB:
                v8B_sb = const.tile([128, TBtot * 8], f32, tag="v8B")

            def side(side_name, lhs_sb, rhs_sb, slot_of, S, val_sb, v8_sb):
                for kk0, gn, direct in geom.groups(side_name):
                    ps = psum.tile([128, GSZ, PSB], f32, tag="ps")
                    for i in range(gn):
                        kk = kk0 + i
                        s = slot_of[kk]
                        nc.tensor.matmul(
                            ps[:, i, 0:W],
                            lhs_sb[:, kk * 128:(kk + 1) * 128],
                            rhs_sb[:, s * W:(s + 1) * W],
                            start=True, stop=True)
                    if direct:
                        for i in range(gn):
                            kk = kk0 + i
                            nc.vector.max(v8_sb[:, kk * 8:(kk + 1) * 8],
                                          ps[:, i, 0:W])
                    else:
                        nc.scalar.activation(
                            S[:, kk0:kk0 + gn, :], ps[:, 0:gn, 0:W],
                            mybir.ActivationFunctionType.Copy)
                        b1 = work.tile([128, GSZ, H1], f16, tag="b1")
                        nc.vector.tensor_tensor(
                            b1[:, 0:gn, :],
                            S[:, kk0:kk0 + gn, 0:H1],
                            S[:, kk0:kk0 + gn, H1:W], op=Alu.max)
                        b2 = work.tile([128, GSZ, H2], f16, tag="b2")
                        nc.vector.tensor_tensor(
                            b2[:, 0:gn, :],
                            b1[:, 0:gn, 0:H2],
                            b1[:, 0:gn, H2:H1], op=Alu.max)
                        nc.vector.tensor_reduce(
                            val_sb[:, kk0:kk0 + gn], b2[:, 0:gn, :],
                            Ax.X, Alu.max)

            side("A", lhsA_sb, rhsB_sb, geom.slotA, SA, valA_sb, v8A_sb)
            side("B", lhsB_sb, rhsA_sb, geom.slotB, SB, valB_sb, v8B_sb)

            nc.sync.dma_start(valA[:], valA_sb[:])
            hb = max(1, TBtot - GSZ)
            nc.sync.dma_start(valB[:, 0:hb], valB_sb[:, 0:hb])
            nc.sync.dma_start(valB[:, hb:], valB_sb[:, hb:])
            if dirA:
                nc.sync.dma_start(v8A[:], v8A_sb[:])
            if dirB:
                nc.sync.dma_start(v8B[:], v8B_sb[:])

    nc.compile()
    return nc


def _pack_side(pos_row, pos_col, starts_row, starts_col, graphs, baseT, W):
    """lhs/rhs bf16 packs for one core, one direction.

    PSUM value = 2 p.q - |q|^2 - |p|^2 = -d^2.
    K-row order: tier-2 (smallest) first, tier-0 last, |p|^2 rows last.
    """
    LT = int(baseT[-1]) * 128
    lhs = np.zeros((K, LT), dtype=np.float32)
    rhs = np.zeros((K, GPC * W), dtype=np.float32)
    #  rows 0-8   : tier2 cross (c,x3) lhs a1,a2,a3 / rhs b3,b2,b1
    #  row  9     : tier2 -q3      (lhs -1, rhs q3)
    #  rows 10-15 : tier1 cross    lhs a1,a2 / rhs b2,b1
    #  row  16    : tier1 -q2
    #  rows 17-19 : tier0 cross    lhs a1 / rhs b1
    #  row  20    : tier0 -q1  (+BIG on padding -> pad col = -BIG)
    #  rows 21-23 : -|p|^2 tiers (lhs p-squared splits, rhs -1)
    lhs[9, :] = -1.0
    lhs[16, :] = -1.0
    lhs[20, :] = -1.0
    rhs[20, :] = BIG
    tb = 0
    for s in range(GPC):
        g = graphs[s]
        p = pos_row[starts_row[g]:starts_row[g + 1]]
        n = p.shape[0]
        lb = int(baseT[s]) * 128
        for c in range(3):
            a1, a2, a3 = _split3(np.float32(2.0) * p[:, c])
            lhs[0 + c * 3, lb:lb + n] = a1
            lhs[1 + c * 3, lb:lb + n] = a2
            lhs[2 + c * 3, lb:lb + n] = a3
            lhs[10 + c * 2, lb:lb + n] = a1
            lhs[11 + c * 2, lb:lb + n] = a2
            lhs[17 + c, lb:lb + n] = a1
        pp = (p[:, 0] * p[:, 0] + p[:, 1] * p[:, 1]) + p[:, 2] * p[:, 2]
        p1, p2, p3 = _split3(pp)
        lhs[21, lb:lb + n] = p3
        lhs[22, lb:lb + n] = p2
        lhs[23, lb:lb + n] = p1

        q = pos_col[starts_col[g]:starts_col[g + 1]]
        m = q.shape[0]
        rb = s * W
        qq = (q[:, 0] * q[:, 0] + q[:, 1] * q[:, 1]) + q[:, 2] * q[:, 2]
        q1, q2, q3 = _split3(qq)
        for c in range(3):
            b1, b2, b3 = _split3(q[:, c])
            rhs[0 + c * 3, rb:rb + m] = b3
            rhs[1 + c * 3, rb:rb + m] = b2
            rhs[2 + c * 3, rb:rb + m] = b1
            rhs[10 + c * 2, rb:rb + m] = b2
            rhs[11 + c * 2, rb:rb + m] = b1
            rhs[17 + c, rb:rb + m] = b1
        rhs[9, rb:rb + m] = q3
        rhs[16, rb:rb + m] = q2
        rhs[20, rb:rb + m] = q1
        rhs[21, rb:rb + m] = -1.0
        rhs[22, rb:rb + m] = -1.0
        rhs[23, rb:rb + m] = -1.0
        tb += 1
    return lhs.astype(BF16), rhs.astype(BF16)


def _unpack_side(val, v8, groups, starts_row, graphs, TA, baseT, dist_full):
    direct_kks = set()
    for kk0, gn, direct in groups:
        if direct:
            direct_kks.update(range(kk0, kk0 + gn))
    for s in range(GPC):
        g = graphs[s]
        n = starts_row[g + 1] - starts_row[g]
        for t in range((n + 127) // 128):
            rows = min(128, n - t * 128)
            kk = int(baseT[s]) + t
            if kk in direct_kks:
                v = v8[:rows, kk * 8].astype(np.float64)
            else:
                v = val[:rows, kk].astype(np.float64)
            atoms = starts_row[g] + t * 128 + np.arange(rows)
            dist_full[atoms] = np.sqrt(np.maximum(-v, 0.0))


def _patch_rows(pos_row, pos_col, starts_row, starts_col, n2g_row,
                dist_full):
    """Exact recompute (reference arithmetic) for rows near the cutoff."""
    sel = np.nonzero(np.abs(dist_full - 10.0) < PATCH_BAND)[0]
    for r in sel:
        g = int(n2g_row[r])
        Q = pos_col[starts_col[g]:starts_col[g + 1]]
        p = pos_row[r]
        d2 = ((p[0] - Q[:, 0]) ** 2 + (p[1] - Q[:, 1]) ** 2
              + (p[2] - Q[:, 2]) ** 2)
        j = int(np.argmin(d2))
        d = p - Q[j]
        dist_full[r] = np.sqrt((d[0] * d[0] + d[1] * d[1]) + d[2] * d[2])


def kernel(pos_a, pos_b, node2graph_a, node2graph_b,
           atom2residue_a, atom2residue_b, is_mutation):
    global LAST_EXEC_NS

    from concourse.bass_utils import run_bass_kernel_spmd

    pos_a = np.asarray(pos_a, dtype=np.float32)
    pos_b = np.asarray(pos_b, dtype=np.float32)
    node2graph_a = np.asarray(node2graph_a)
    node2graph_b = np.asarray(node2graph_b)
    atom2residue_a = np.asarray(atom2residue_a)
    atom2residue_b = np.asarray(atom2residue_b)
    is_mutation = np.asarray(is_mutation)

    Na = pos_a.shape[0]
    Nb = pos_b.shape[0]

    sa = np.searchsorted(node2graph_a, np.arange(G + 1)).astype(np.int64)
    sb = np.searchsorted(node2graph_b, np.arange(G + 1)).astype(np.int64)
    na = np.diff(sa)
    nb = np.diff(sb)
    assert na.min() > 0 and nb.min() > 0, "empty graph block not supported"

    geom = _Geom(na, nb)
    key = geom.key()
    if key not in _prog_cache:
        _prog_cache[key] = _build_program(geom)
    nc = _prog_cache[key]

    in_maps = []
    for c in range(NCORES):
        lhsA, rhsB = _pack_side(pos_a, pos_b, sa, sb, geom.graphA[c],
                                geom.baseTA, geom.WSTAR)
        lhsB, rhsA = _pack_side(pos_b, pos_a, sb, sa, geom.graphB[c],
                                geom.baseTB, geom.WSTAR)
        in_maps.append({"lhsA": lhsA, "rhsB": rhsB,
                        "lhsB": lhsB, "rhsA": rhsA})

    if PROFILE:
        _install_ntff_hook()
    res = run_bass_kernel_spmd(nc, in_maps, list(range(NCORES)),
                               trace=bool(PROFILE))
    if PROFILE:
        LAST_EXEC_NS = res.exec_time_ns

    dist_a = np.zeros(Na, dtype=np.float64)
    dist_b = np.zeros(Nb, dtype=np.float64)
    gA = geom.groups("A")
    gB = geom.groups("B")
    for c in range(NCORES):
        _unpack_side(res.results[c]["valA"], res.results[c].get("v8A"), gA,
                     sa, geom.graphA[c], geom.TA, geom.baseTA, dist_a)
        _unpack_side(res.results[c]["valB"], res.results[c].get("v8B"), gB,
                     sb, geom.graphB[c], geom.TB, geom.baseTB, dist_b)

    dist_a = dist_a.astype(np.float32)
    dist_b = dist_b.astype(np.float32)
    _patch_rows(pos_a, pos_b, sa, sb, node2graph_a, dist_a)
    _patch_rows(pos_b, pos_a, sb, sa, node2graph_b, dist_b)

    def iface_mask(dist, atom2residue):
        is_if = (dist < CUTOFF).astype(np.int32)
        res_max = np.zeros(NUM_RESIDUES, dtype=np.int32)
        np.maximum.at(res_max, atom2residue, is_if)
        return res_max[atom2residue] > 0

    mask_a = iface_mask(dist_a, atom2residue_a)
    mask_b = iface_mask(dist_b, atom2residue_b)
    mask = np.concatenate([mask_a, mask_b]) | is_mutation.astype(bool)
    dists = np.concatenate([dist_a, dist_b]).astype(np.float32)
    return mask, dists
